# revision 38
# baseline (speedup 1.0000x reference)
"""GQA attention (b=2, s=2048, d=2048, H=16, Hkv=4, depth=128) on 8 trn2 cores.

Sharding: core c = 4*b + j (b in {0,1}, j in {0..3}) handles batch b and
q-heads {2j, 2j+1, 2j+8, 2j+9}.  This model's RoPE rotates the full projected
vector (pairing dim i with i + d/2), so roped q-head h mixes raw column
blocks {h mod 8, (h mod 8) + 8}; the head grouping above makes the Wq column
shard exactly 512 columns with no duplication.  Those q-heads attend kv-heads
{g0, g0+2} (g0 = 0 for j<2 else 1), which likewise pair up under RoPE.
Each core of a pair projects ONE raw k block and ONE v head; the pair swaps
them with a 2-way AllGather, halving the duplicated K/V projection work.
Wo is row-sharded over the 4 local head-dims; the 4 per-batch bf16 partials
are summed on the host (fp32) and bo added.

Device layout is fully transposed (feature dim on partitions): q_r^T, k_r^T
are [depth, s]; logits are computed as l^T = k_r^T.T @ q_r^T so the softmax
free axis is sq and the PV matmul needs no transposes (v is kept native
[s, dv] via on-chip DMA transposes).  All matmuls run in bf16 (fp32 PSUM).
Softmax denominators: bf16 pair-tree partial sums on DVE, cross-partition
ones-matmul reduce + broadcast on PE (back to back, bf16), and 1/d applied
as exp(-ln(d)) on the scalar engine.  exp() drains two QK PSUM banks per
ACT op ([128,1024]).  Inputs stream per k-chunk so the projection
accumulation pipelines with the DMAs.
"""
import numpy as np
import ml_dtypes
from contextlib import ExitStack

import concourse.bass as bass
import concourse.mybir as mybir
import concourse.tile as tile
from concourse.bass import ts
from concourse.bass_utils import run_bass_kernel_spmd
from concourse.masks import make_identity

BF = mybir.dt.bfloat16
F32 = mybir.dt.float32
F32R = mybir.dt.float32r
NPBF = ml_dtypes.bfloat16

S = 2048          # sequence length
D = 2048          # d_model
DEPTH = 128       # head dim
NKC = 16          # contraction chunks of 128 over d_model
NST = 4           # 512-wide s tiles
INV_SQRT_D = 1.0 / float(np.sqrt(np.float32(DEPTH)))

_NC_CACHE = None
LAST_RESULT = None  # BassKernelResults of the most recent run (for profiling)


def _split_waits(nc, limit=1):
    """walrus rejects instructions carrying more than a couple of sem waits
    ('Too many sync wait commands').  Move excess waits onto dedicated NoOps
    on the same engine, placed immediately before the instruction."""
    idx = 0
    for f in nc.m.functions:
        for blk in f.blocks:
            insts = blk.instructions
            out = []
            for inst in insts:
                si = inst.sync_info
                if si is not None and len(si.on_wait) > limit:
                    waits = list(si.on_wait)
                    extra, keep = waits[:-limit], waits[-limit:]
                    for w in extra:
                        nop = mybir.InstNoOp(name=f"waitsplit_{idx}", ins=[], outs=[])
                        idx += 1
                        nop.engine = inst.engine
                        nop.bass_nofuse = True
                        nop.sync_info = mybir.SyncInfo(on_wait=[w], on_update=[])
                        out.append(nop)
                    inst.sync_info = mybir.SyncInfo(
                        on_wait=keep, on_update=list(si.on_update)
                    )
                out.append(inst)
            insts[:] = out


def _ap_sig(arg):
    """Signature of a lowered AP argument for LDW dedup."""
    try:
        t = arg.tensor_name if hasattr(arg, "tensor_name") else getattr(arg, "name", None)
        return (str(t), str(getattr(arg, "offset", None)), str(getattr(arg, "ap", None)),
                str(getattr(arg, "dtype", None)))
    except Exception:
        return None


def _dedup_ldweights(nc):
    """Replace InstLdweights that reload the exact same stationary operand
    (with only Matmults in between on PE) with NoOps carrying the same name,
    waits and updates.  Equivalent to walrus's disabled --enable-ldw-opt, but
    done only for provably-identical consecutive loads."""
    n_dedup = 0
    for f in nc.m.functions:
        for blk in f.blocks:
            insts = blk.instructions
            last_sig = None
            for idx, inst in enumerate(insts):
                eng = str(inst.engine)
                if not eng.endswith("PE"):
                    continue
                nm = type(inst).__name__
                if nm == "InstLdweights":
                    if getattr(inst, "is_transpose", None):
                        last_sig = None
                        continue
                    sig = _ap_sig(inst.ins[0]) if inst.ins else None
                    if sig is not None and sig == last_sig:
                        nop = mybir.InstNoOp(name=inst.name, ins=[], outs=[])
                        nop.engine = inst.engine
                        nop.bass_nofuse = True
                        if inst.sync_info is not None:
                            nop.sync_info = mybir.SyncInfo(
                                on_wait=list(inst.sync_info.on_wait),
                                on_update=list(inst.sync_info.on_update),
                            )
                        try:
                            nop.set_dependency_edges(inst.dependency_edges)
                        except Exception:
                            pass
                        insts[idx] = nop
                        n_dedup += 1
                    else:
                        last_sig = sig
                elif nm == "InstMatmult":
                    if getattr(inst, "is_transpose", None):
                        last_sig = None
                    continue
                else:
                    last_sig = None
    return n_dedup


def _build_nc():
    nc = bass.Bass(num_devices=8)
    xT = nc.dram_tensor("xT", [128, NKC, S], BF, kind="ExternalInput")
    wq = nc.dram_tensor("wq", [128, NKC, 512], BF, kind="ExternalInput")
    wk = nc.dram_tensor("wk", [128, NKC, 128], BF, kind="ExternalInput")
    wv = nc.dram_tensor("wv", [128, NKC, 128], BF, kind="ExternalInput")
    wo = nc.dram_tensor("wo", [128, 4, D], BF, kind="ExternalInput")
    cq = nc.dram_tensor("cq", [128, 2, S], BF, kind="ExternalInput")
    sq = nc.dram_tensor("sq", [128, 2, S], BF, kind="ExternalInput")
    ck = nc.dram_tensor("ck", [128, S], BF, kind="ExternalInput")
    sk = nc.dram_tensor("sk", [128, S], BF, kind="ExternalInput")
    out = nc.dram_tensor("out", [128, 16, D], BF, kind="ExternalOutput")
    scratch = nc.dram_tensor("oscratch", [128, 16, D], BF)

    with tile.TileContext(nc) as tc, ExitStack() as top:
        pool_p = top.enter_context(tc.tile_pool(name="persist", bufs=1))
        pp = top.enter_context(tc.tile_pool(name="psum", bufs=4, space="PSUM"))
        pp2 = top.enter_context(tc.tile_pool(name="psum2", bufs=2, space="PSUM"))
        pool_small = top.enter_context(tc.tile_pool(name="small", bufs=4))

        qr = pool_p.tile([128, 4, S], BF)        # roped qT, slots [a0,a1,a0+8,a1+8]
        kr = pool_p.tile([128, 2, S], BF)        # roped kT,  slots [g0, g0+2]
        vn = pool_p.tile([128, 2, NKC, DEPTH], BF)  # v native [p, g, skc, dv]
        ones_col_b = pool_p.tile([128, 1], BF)
        ones_row_b = pool_p.tile([1, 128], BF)
        ident = pool_p.tile([128, 128], BF)
        nc.vector.memset(ones_col_b[:], 1.0)
        nc.vector.memset(ones_row_b[:], 1.0)
        make_identity(nc, ident[:])

        # ---------------- phase 1: projections + rope -----------------
        with ExitStack() as p1:
            pool_x = p1.enter_context(tc.tile_pool(name="p1x", bufs=16))
            pool_w = p1.enter_context(tc.tile_pool(name="p1w", bufs=16))
            pool_tab = p1.enter_context(tc.tile_pool(name="p1t", bufs=1))
            pool_t = p1.enter_context(tc.tile_pool(name="p1tmp", bufs=4))
            pool_vt = p1.enter_context(tc.tile_pool(name="p1vt", bufs=1))
            pool_dram = p1.enter_context(tc.tile_pool(name="p1dram", bufs=1, space="DRAM"))

            # per-chunk tiles so the contraction pipeline starts as soon
            # as the first k-chunks of x^T / W land (instead of waiting for
            # one monolithic 8 MB DMA)
            xTs, wqs, wks, wvs = [], [], [], []
            for kc in range(NKC):
                xt_t = pool_x.tile([128, S], BF, tag="xt", name=f"xt_{kc}")
                nc.sync.dma_start(xt_t[:], xT[:, kc, :])
                xTs.append(xt_t)
                wk_t = pool_w.tile([128, 128], BF, tag="wk", name=f"wk_{kc}")
                nc.sync.dma_start(wk_t[:], wk[:, kc, :])
                wks.append(wk_t)
                wv_t = pool_w.tile([128, 128], BF, tag="wv", name=f"wv_{kc}")
                nc.sync.dma_start(wv_t[:], wv[:, kc, :])
                wvs.append(wv_t)
            for kc in range(NKC):
                wq_t = pool_w.tile([128, 512], BF, tag="wq", name=f"wq_{kc}")
                nc.sync.dma_start(wq_t[:], wq[:, kc, :])
                wqs.append(wq_t)
            cq_sb = pool_tab.tile([128, 2, S], BF)
            sq_sb = pool_tab.tile([128, 2, S], BF)
            nc.sync.dma_start(cq_sb[:, 0, :], cq[:, 0, :])
            nc.sync.dma_start(sq_sb[:, 0, :], sq[:, 0, :])
            ck_sb = pool_tab.tile([128, S], BF)
            nc.sync.dma_start(ck_sb[:], ck[:])
            sk_sb = pool_tab.tile([128, S], BF)
            nc.sync.dma_start(sk_sb[:], sk[:])
            nc.sync.dma_start(cq_sb[:, 1, :], cq[:, 1, :])
            nc.sync.dma_start(sq_sb[:, 1, :], sq[:, 1, :])

            def proj_pair_rope(w_sb, nblk, i, st, c_ap, s_ap, out1, out2):
                """raw blocks (i, nblk+i) of w_sb projected over st, roped into
                out1 (x1*c - x2*s) and out2 (x2*c + x1*s)."""
                raws = []
                for xb in range(2):
                    blk = i if xb == 0 else nblk + i
                    acc = pp.tile([128, 512], F32, tag="ps")
                    for kc in range(NKC):
                        nc.tensor.matmul(
                            acc[:],
                            w_sb[kc][:, ts(blk, 128)],
                            xTs[kc][:, ts(st, 512)],
                            start=(kc == 0),
                            stop=(kc == NKC - 1),
                        )
                    raw = pool_t.tile([128, 512], BF, tag="raw")
                    nc.scalar.copy(raw[:], acc[:])
                    raws.append(raw)
                x1, x2 = raws
                t1 = pool_t.tile([128, 512], BF, tag="t1")
                t2 = pool_t.tile([128, 512], BF, tag="t2")
                nc.vector.tensor_mul(t1[:], x1[:], c_ap)
                nc.vector.tensor_mul(t2[:], x2[:], s_ap)
                nc.vector.tensor_sub(out1, t1[:], t2[:])
                t3 = pool_t.tile([128, 512], BF, tag="t1")
                t4 = pool_t.tile([128, 512], BF, tag="t2")
                nc.vector.tensor_mul(t3[:], x2[:], c_ap)
                nc.vector.tensor_mul(t4[:], x1[:], s_ap)
                nc.vector.tensor_add(out2, t3[:], t4[:])

            # K/V: each core of a pair projects ONE raw k block and ONE v
            # head; the pair exchanges them with an AllGather, then ropes /
            # transposes locally.  Halves the duplicated K/V projection work.
            kv_sb = pool_vt.tile([128, 2 * S], BF, tag="kvmine")
            for part, w_list in ((0, wks), (1, wvs)):
                for st in range(NST):
                    acc = pp.tile([128, 512], F32, tag="ps")
                    for kc in range(NKC):
                        nc.tensor.matmul(
                            acc[:],
                            w_list[kc][:],
                            xTs[kc][:, ts(st, 512)],
                            start=(kc == 0),
                            stop=(kc == NKC - 1),
                        )
                    nc.scalar.copy(kv_sb[:, ts(part * NST + st, 512)], acc[:])
            kv_in = pool_dram.tile([128, 2 * S], BF)
            kv_out = pool_dram.tile([2, 128, 2 * S], BF)
            nc.sync.dma_start(kv_in[:], kv_sb[:])
            nc.gpsimd.collective_compute(
                "AllGather",
                mybir.AluOpType.bypass,
                replica_groups=[[0, 1], [2, 3], [4, 5], [6, 7]],
                ins=[kv_in.opt()],
                outs=[kv_out.opt()],
            )
            kboth = pool_vt.tile([128, 2, S], BF, tag="kboth")
            vtboth = pool_vt.tile([128, 2, S], BF, tag="vtboth")
            for r in range(2):
                nc.sync.dma_start(kboth[:, r, :], kv_out[r, :, 0:S])
                nc.sync.dma_start(vtboth[:, r, :], kv_out[r, :, S:2 * S])

            # Q: pairs (i, 2+i) -> qr slots (i, 2+i)
            for i in range(2):
                for st in range(NST):
                    proj_pair_rope(
                        wqs, 2, i, st,
                        cq_sb[:, i, ts(st, 512)], sq_sb[:, i, ts(st, 512)],
                        qr[:, i, ts(st, 512)], qr[:, 2 + i, ts(st, 512)],
                    )

            # k rope from the gathered raw blocks (x1 = even core's block g0,
            # x2 = odd core's block g0+2)
            for st in range(NST):
                sl = ts(st, 512)
                x1, x2 = kboth[:, 0, sl], kboth[:, 1, sl]
                c_ap, s_ap = ck_sb[:, sl], sk_sb[:, sl]
                t1 = pool_t.tile([128, 512], BF, tag="t1")
                t2 = pool_t.tile([128, 512], BF, tag="t2")
                nc.vector.tensor_mul(t1[:], x1, c_ap)
                nc.vector.tensor_mul(t2[:], x2, s_ap)
                nc.vector.tensor_sub(kr[:, 0, sl], t1[:], t2[:])
                t3 = pool_t.tile([128, 512], BF, tag="t1")
                t4 = pool_t.tile([128, 512], BF, tag="t2")
                nc.vector.tensor_mul(t3[:], x2, c_ap)
                nc.vector.tensor_mul(t4[:], x1, s_ap)
                nc.vector.tensor_add(kr[:, 1, sl], t3[:], t4[:])

            # v native via DMA transpose
            for g in range(2):
                for skt in range(NKC):
                    nc.sync.dma_start_transpose(
                        vn[:, g, skt, :], vtboth[:, g, ts(skt, 128)]
                    )

        # ------------- phase 2: attention + output projection -------------
        with ExitStack() as p2:
            pool_exp = p2.enter_context(tc.tile_pool(name="exp", bufs=12))
            pool_sums = p2.enter_context(tc.tile_pool(name="sums", bufs=8))
            pool_wo = p2.enter_context(tc.tile_pool(name="wop", bufs=1))
            pool_on = p2.enter_context(tc.tile_pool(name="onorm", bufs=1))
            pool_bc = p2.enter_context(tc.tile_pool(name="bcast", bufs=6))
            pool_out = p2.enter_context(tc.tile_pool(name="osb", bufs=3))

            onorm = pool_on.tile([128, 4, S], BF)
            wo_sb = pool_wo.tile([128, 4, D], BF)
            nc.sync.dma_start(wo_sb[:], wo[:])

            def attend(hi):
                g = hi // 2
                for half in range(2):
                    sts = (2 * half, 2 * half + 1)
                    o_banks = {
                        st: pp.tile([128, 512], F32, tag="ps",
                                    name=f"ob_{hi}_{st}")
                        for st in sts
                    }
                    sums = {
                        st: pool_sums.tile([128, 512], F32, tag="sums",
                                           name=f"sum_{hi}_{st}")
                        for st in sts
                    }
                    pairs = {
                        st: [pool_sums.tile([128, 512], BF, tag="pair",
                                            name=f"pr_{hi}_{st}_{k}")
                             for k in range(NKC // 2)]
                        for st in sts
                    }
                    prev = {}
                    for skt in range(NKC):
                        lg2 = pp2.tile([128, 1024], F32, tag="ps2")
                        e = pool_exp.tile([128, 1024], BF, tag="exp")
                        for idx, st in enumerate(sts):
                            nc.tensor.matmul(
                                lg2[:, ts(idx, 512)],
                                kr[:, g, ts(skt, 128)],
                                qr[:, hi, ts(st, 512)],
                                start=True, stop=True,
                            )
                        nc.scalar.activation(
                            e[:], lg2[:],
                            mybir.ActivationFunctionType.Exp,
                            scale=INV_SQRT_D,
                        )
                        for idx, st in enumerate(sts):
                            if skt % 2 == 0:
                                prev[st] = e[:, ts(idx, 512)]
                            else:
                                k = skt // 2
                                nc.vector.tensor_add(
                                    pairs[st][k][:], prev[st],
                                    e[:, ts(idx, 512)]
                                )
                                # fold the pair into the running sum right
                                # away so denominators are ready when the
                                # last chunk lands (no end-of-head latency)
                                if k == 0:
                                    nc.vector.tensor_copy(sums[st][:],
                                                          pairs[st][0][:])
                                else:
                                    nc.vector.tensor_add(sums[st][:],
                                                         sums[st][:],
                                                         pairs[st][k][:])
                        for idx, st in enumerate(sts):
                            nc.tensor.matmul(
                                o_banks[st][:],
                                vn[:, g, skt, :],
                                e[:, ts(idx, 512)],
                                start=(skt == 0),
                                stop=(skt == NKC - 1),
                            )
                    # denominator (bf16) -> broadcast (both PE, back to
                    # back) -> 1/d as exp(-ln(d)) on ACT -> DVE scale of o^T
                    for st in sts:
                        sums_bf = pool_small.tile([128, 512], BF, tag="sumbf")
                        nc.vector.tensor_copy(sums_bf[:], sums[st][:])
                        den = pp.tile([1, 512], F32, tag="ps")
                        nc.tensor.matmul(den[:], ones_col_b[:], sums_bf[:],
                                         start=True, stop=True)
                        den_bf = pool_small.tile([1, 512], BF, tag="denbf")
                        nc.vector.tensor_copy(den_bf[:], den[:])
                        bc_ps = pp.tile([128, 512], F32, tag="ps")
                        nc.tensor.matmul(
                            bc_ps[:],
                            ones_row_b[:],
                            den_bf[:],
                            start=True, stop=True,
                        )
                        lbc = pool_bc.tile([128, 512], F32, tag="lbc")
                        nc.scalar.activation(lbc[:], bc_ps[:],
                                             mybir.ActivationFunctionType.Ln)
                        bc_sb = pool_bc.tile([128, 512], F32, tag="bc")
                        nc.scalar.activation(bc_sb[:], lbc[:],
                                             mybir.ActivationFunctionType.Exp,
                                             scale=-1.0)
                        nc.vector.tensor_mul(
                            onorm[:, hi, ts(st, 512)], o_banks[st][:], bc_sb[:]
                        )

            attend(0)
            attend(2)
            attend(1)
            attend(3)

            # output projection: out[sq, n] += onorm_h^T.T @ wo_h
            for m in range(16):
                obanks = [pp.tile([128, 512], F32, tag="ps", name=f"op_{m}_{i}")
                          for i in range(4)]
                for hi in range(4):
                    for ct in range(4):
                        nc.tensor.matmul(
                            obanks[ct][:],
                            onorm[:, hi, ts(m, 128)],
                            wo_sb[:, hi, ts(ct, 512)],
                            start=(hi == 0),
                            stop=(hi == 3),
                        )
                o_sb = pool_out.tile([128, D], BF, tag="out")
                for ct in range(4):
                    if ct % 2 == 0:
                        nc.vector.tensor_copy(o_sb[:, ts(ct, 512)], obanks[ct][:])
                    else:
                        nc.scalar.copy(o_sb[:, ts(ct, 512)], obanks[ct][:])
                nc.sync.dma_start(out[:, m, :], o_sb[:])

    import os
    if os.environ.get("BASS_LDW_DEDUP", "0") == "1":
        n = _dedup_ldweights(nc)
    _split_waits(nc)
    return nc


def _chunk128(arr):
    """(K*128, N) f32 -> [128, K, N] bf16 with [p, k, n] = arr[k*128+p, n]."""
    k = arr.shape[0] // 128
    return np.ascontiguousarray(
        arr.reshape(k, 128, arr.shape[1]).transpose(1, 0, 2)
    ).astype(NPBF)


def _rope_tables(dim):
    pos = np.arange(S, dtype=np.float32)
    inv = (10000.0 ** (-(np.arange(dim, dtype=np.float32)) / np.float32(dim))
           ).astype(np.float32)
    freqs = pos[:, None] * inv[None, :]
    return np.cos(freqs).astype(np.float32), np.sin(freqs).astype(np.float32)


def kernel(x, mask, Wq, Wk, Wv, Wo, bo):
    global _NC_CACHE
    assert np.asarray(mask).all(), "kernel specialized for all-true mask"
    x = np.asarray(x, dtype=np.float32)
    Wq = np.asarray(Wq, dtype=np.float32)
    Wk = np.asarray(Wk, dtype=np.float32)
    Wv = np.asarray(Wv, dtype=np.float32)
    Wo = np.asarray(Wo, dtype=np.float32)
    bo = np.asarray(bo, dtype=np.float32)

    cos_q, sin_q = _rope_tables(1024)
    cos_k, sin_k = _rope_tables(256)

    def blk(a, i):  # column block i (width 128) of a
        return a[:, i * 128:(i + 1) * 128]

    in_maps = []
    for c in range(8):
        b, j = c // 4, c % 4
        a0, a1 = 2 * j, 2 * j + 1
        g0 = 0 if j < 2 else 1

        xb = x[b]                                   # (S, D)
        xT3 = _chunk128(np.ascontiguousarray(xb.T))  # [128, 16, S]

        wq_sel = np.concatenate(
            [blk(Wq, a0), blk(Wq, a1), blk(Wq, a0 + 8), blk(Wq, a1 + 8)], axis=1)
        myblk = g0 + 2 * (j % 2)
        wk_sel = blk(Wk, myblk)
        wv_sel = blk(Wv, myblk)
        wo_sel = np.concatenate(
            [Wo[h * 128:(h + 1) * 128, :] for h in (a0, a1, a0 + 8, a1 + 8)],
            axis=0)

        cq_sel = _chunk128(np.ascontiguousarray(
            np.concatenate([blk(cos_q, a0), blk(cos_q, a1)], axis=1).T))
        sq_sel = _chunk128(np.ascontiguousarray(
            np.concatenate([blk(sin_q, a0), blk(sin_q, a1)], axis=1).T))
        ck_sel = np.ascontiguousarray(blk(cos_k, g0).T).astype(NPBF)
        sk_sel = np.ascontiguousarray(blk(sin_k, g0).T).astype(NPBF)

        in_maps.append({
            "xT": xT3,
            "wq": _chunk128(wq_sel),
            "wk": _chunk128(wk_sel),
            "wv": _chunk128(wv_sel),
            "wo": _chunk128(wo_sel),
            "cq": cq_sel, "sq": sq_sel, "ck": ck_sel, "sk": sk_sel,
        })

    global LAST_RESULT
    if _NC_CACHE is None:
        _NC_CACHE = _build_nc()
    res = run_bass_kernel_spmd(_NC_CACHE, in_maps, list(range(8)))
    LAST_RESULT = res

    partials = [
        res.results[c]["out"].astype(np.float32).transpose(1, 0, 2).reshape(S, D)
        for c in range(8)
    ]
    out = np.stack(
        [sum(partials[4 * b + j] for j in range(4)) for b in range(2)], axis=0
    )
    return (out + bo).astype(np.float32)
